# revision 3
# baseline (speedup 1.0000x reference)
"""Trainium2 Bass kernel: per-cluster PCA geometry features (segment reduce).

Problem: data [4194304, 6] f32, clusts [32768, 128] int — per cluster of 128
voxels compute: center (mean of xyz), normalized covariance B = A/lmax,
principal axis v0 scaled by dirwt = 1 - lmid/lmax with a sign fix, size.

Strategy: shard the 32768 clusters across 8 NeuronCores (4096 each). Host
pre-partitions voxel coordinates per cluster (a pure permutation). On device:
  phase 1: per-cluster sums/second moments via free-dim reduces (DVE) with
           product planes on ACT (squares) / GPSIMD (cross products).
  eigen:   batched analytic 3x3 symmetric eigensolve on [128, 32] tiles
           (trig method: Arctan+Sin on ACT), eigenvector via cross products.
  phase 2: second pass over voxel planes for the sign criterion
           sc = sum_s x0 * ||xc - x0 v0||.
Cluster c <-> (partition p = c // 32, segment j = c % 32); voxel planes live
as [128, 32, 128] SBUF tiles, per-cluster scalars as [128, 32] tiles
broadcast into plane ops via stride-0 access patterns.
"""
import numpy as np
from contextlib import ExitStack

import concourse.bass as bass
import concourse.bacc as bacc
import concourse.tile as tile
from concourse import mybir
from concourse.bass_utils import run_bass_kernel_spmd

N_CLUSTS = 32768
CLUST_SIZE = 128
N_CORES = 8
C_LOC = N_CLUSTS // N_CORES   # 4096 clusters per core
P = 128                       # SBUF partitions
NSEG = C_LOC // P             # 32 clusters (segments) per partition
V = CLUST_SIZE                # 128 voxels per cluster

F32 = mybir.dt.float32
AF = mybir.ActivationFunctionType
OP = mybir.AluOpType
AX = mybir.AxisListType

TWO_PI_3 = 2.0943951023931953   # 2*pi/3
PI_3 = 1.0471975511965976       # pi/3

_CACHED = {}
PROFILE = False          # set by test.py to capture an NTFF trace
LAST_RESULT = None       # BassKernelResults of the last run when PROFILE


def _bcast(t):
    """[P, NSEG] per-cluster tile -> [P, NSEG, V] stride-0 broadcast AP."""
    return t[:, :, None].broadcast_to([P, NSEG, V])


def build_nc():
    nc = bacc.Bacc()
    x_d = nc.dram_tensor("x", [C_LOC, V], F32, kind="ExternalInput").ap()
    y_d = nc.dram_tensor("y", [C_LOC, V], F32, kind="ExternalInput").ap()
    z_d = nc.dram_tensor("z", [C_LOC, V], F32, kind="ExternalInput").ap()
    # voxel-major copies: phase-1 segment sums run on the PE (lhsT = plane
    # chunk, rhs = ones), which contracts over partitions = voxel slots
    xt_d = nc.dram_tensor("xt", [V, C_LOC], F32, kind="ExternalInput").ap()
    yt_d = nc.dram_tensor("yt", [V, C_LOC], F32, kind="ExternalInput").ap()
    zt_d = nc.dram_tensor("zt", [V, C_LOC], F32, kind="ExternalInput").ap()
    feats_d = nc.dram_tensor("feats", [C_LOC, 16], F32, kind="ExternalOutput").ap()

    with tile.TileContext(nc) as tc, ExitStack() as ctx:
        pool = ctx.enter_context(tc.tile_pool(name="main", bufs=1))
        # recycled full-plane scratch (16KB/partition each)
        sp = ctx.enter_context(tc.tile_pool(name="scratch", bufs=4))
        pp = ctx.enter_context(tc.tile_pool(name="psum", bufs=1, space="PSUM"))

        def plane(name):
            return sp.tile([P, NSEG, V], F32, tag="plane", name=name)

        def small(name, pool_=None):
            return (pool_ or pool).tile([P, NSEG], F32, tag=f"s_{name}", name=name)

        # bias constants for Sin activations (activation bias must be an SBUF AP)
        bias_pi2 = pool.tile([P, 1], F32, tag="bias_pi2")
        bias_pi6 = pool.tile([P, 1], F32, tag="bias_pi6")
        nc.gpsimd.memset(bias_pi2[:], 1.5707963267948966)
        nc.gpsimd.memset(bias_pi6[:], 0.5235987755982988)

        Xt = pool.tile([P, C_LOC], F32, tag="Xt")
        Yt = pool.tile([P, C_LOC], F32, tag="Yt")
        Zt = pool.tile([P, C_LOC], F32, tag="Zt")
        nc.sync.dma_start(Xt[:], xt_d)
        nc.sync.dma_start(Yt[:], yt_d)
        nc.sync.dma_start(Zt[:], zt_d)

        X = pool.tile([P, NSEG, V], F32, tag="X")
        Y = pool.tile([P, NSEG, V], F32, tag="Y")
        Z = pool.tile([P, NSEG, V], F32, tag="Z")
        nc.sync.dma_start(X[:], x_d.rearrange("(p s) v -> p s v", p=P))
        nc.sync.dma_start(Y[:], y_d.rearrange("(p s) v -> p s v", p=P))
        nc.sync.dma_start(Z[:], z_d.rearrange("(p s) v -> p s v", p=P))

        ones = pool.tile([P, 1], F32, tag="ones")
        nc.gpsimd.memset(ones[:], 1.0)

        # ---------------- phase 1: sums + second moments on the PE ----------
        # psum[:, k*NSEG + j] = sum over voxels of plane k, cluster chunk j
        psum = pp.tile([P, 9 * NSEG], F32, tag="psums")

        def pe_colsums(plane_t, k):
            for j in range(NSEG):
                nc.tensor.matmul(
                    out=psum[:, k * NSEG + j : k * NSEG + j + 1],
                    lhsT=plane_t[:, j * P : (j + 1) * P],
                    rhs=ones[:, 0:1],
                    start=True,
                    stop=True,
                )

        pe_colsums(Xt, 0)
        pe_colsums(Yt, 1)
        pe_colsums(Zt, 2)
        sqt = sp.tile([P, C_LOC], F32, tag="plane", name="sqt")
        nc.scalar.activation(sqt[:], Xt[:], AF.Square)
        pe_colsums(sqt, 3)
        sqt2 = sp.tile([P, C_LOC], F32, tag="plane", name="sqt2")
        nc.scalar.activation(sqt2[:], Yt[:], AF.Square)
        pe_colsums(sqt2, 4)
        sqt3 = sp.tile([P, C_LOC], F32, tag="plane", name="sqt3")
        nc.scalar.activation(sqt3[:], Zt[:], AF.Square)
        pe_colsums(sqt3, 5)
        prt = sp.tile([P, C_LOC], F32, tag="plane", name="prt")
        nc.gpsimd.tensor_tensor(prt[:], Xt[:], Yt[:], OP.mult)
        pe_colsums(prt, 6)
        prt2 = sp.tile([P, C_LOC], F32, tag="plane", name="prt2")
        nc.gpsimd.tensor_tensor(prt2[:], Xt[:], Zt[:], OP.mult)
        pe_colsums(prt2, 7)
        prt3 = sp.tile([P, C_LOC], F32, tag="plane", name="prt3")
        nc.gpsimd.tensor_tensor(prt3[:], Yt[:], Zt[:], OP.mult)
        pe_colsums(prt3, 8)

        moments = pool.tile([P, 9, NSEG], F32, tag="moments")
        nc.vector.tensor_copy(moments[:], psum[:].rearrange("p (k s) -> p k s", k=9))
        Sx = moments[:, 0]; Sy = moments[:, 1]; Sz = moments[:, 2]
        Mxx = moments[:, 3]; Myy = moments[:, 4]; Mzz = moments[:, 5]
        Mxy = moments[:, 6]; Mxz = moments[:, 7]; Myz = moments[:, 8]

        # ---------------- phase 1.5: A matrix, eigensolve ----------------
        # helpers for tiny [P, NSEG] ops
        def tt(eng, out, a, b, op):
            eng.tensor_tensor(out[:], a[:], b[:], op)

        def act(out, in_, func, bias=0.0, scale=1.0):
            nc.scalar.activation(out[:], in_[:], func, bias=bias, scale=scale)

        inv_s = 1.0 / V
        cx = small("cx"); cy = small("cy"); cz = small("cz")
        nc.vector.tensor_scalar(out=cx[:], in0=Sx[:], scalar1=inv_s, scalar2=None, op0=OP.mult)
        nc.vector.tensor_scalar(out=cy[:], in0=Sy[:], scalar1=inv_s, scalar2=None, op0=OP.mult)
        nc.vector.tensor_scalar(out=cz[:], in0=Sz[:], scalar1=inv_s, scalar2=None, op0=OP.mult)

        # centered second moments: a_ij = M_ij - S_i * S_j / V
        axx = small("axx"); ayy = small("ayy"); azz = small("azz")
        axy = small("axy"); axz = small("axz"); ayz = small("ayz")
        t0 = small("t0"); t1 = small("t1"); t2 = small("t2"); t3 = small("t3")
        act(t0, Sx, AF.Square)
        nc.vector.scalar_tensor_tensor(out=axx[:], in0=t0[:], scalar=-inv_s, in1=Mxx[:], op0=OP.mult, op1=OP.add)
        act(t1, Sy, AF.Square)
        nc.vector.scalar_tensor_tensor(out=ayy[:], in0=t1[:], scalar=-inv_s, in1=Myy[:], op0=OP.mult, op1=OP.add)
        act(t2, Sz, AF.Square)
        nc.vector.scalar_tensor_tensor(out=azz[:], in0=t2[:], scalar=-inv_s, in1=Mzz[:], op0=OP.mult, op1=OP.add)
        tt(nc.gpsimd, t0, Sx, Sy, OP.mult)
        nc.vector.scalar_tensor_tensor(out=axy[:], in0=t0[:], scalar=-inv_s, in1=Mxy[:], op0=OP.mult, op1=OP.add)
        tt(nc.gpsimd, t1, Sx, Sz, OP.mult)
        nc.vector.scalar_tensor_tensor(out=axz[:], in0=t1[:], scalar=-inv_s, in1=Mxz[:], op0=OP.mult, op1=OP.add)
        tt(nc.gpsimd, t2, Sy, Sz, OP.mult)
        nc.vector.scalar_tensor_tensor(out=ayz[:], in0=t2[:], scalar=-inv_s, in1=Myz[:], op0=OP.mult, op1=OP.add)

        # q = tr(A)/3 ; b_ii = a_ii - q
        q = small("q")
        tt(nc.vector, t0, axx, ayy, OP.add)
        tt(nc.vector, t0, t0, azz, OP.add)
        nc.vector.tensor_scalar(out=q[:], in0=t0[:], scalar1=1.0 / 3.0, scalar2=None, op0=OP.mult)
        b11 = small("b11"); b22 = small("b22"); b33 = small("b33")
        tt(nc.vector, b11, axx, q, OP.subtract)
        tt(nc.vector, b22, ayy, q, OP.subtract)
        tt(nc.vector, b33, azz, q, OP.subtract)

        # p2 = b11^2+b22^2+b33^2 + 2*(axy^2+axz^2+ayz^2); p = sqrt(p2/6)
        p2 = small("p2")
        act(t0, b11, AF.Square)
        act(t1, b22, AF.Square)
        act(t2, b33, AF.Square)
        tt(nc.vector, t0, t0, t1, OP.add)
        tt(nc.vector, t0, t0, t2, OP.add)
        act(t1, axy, AF.Square)
        act(t2, axz, AF.Square)
        act(t3, ayz, AF.Square)
        tt(nc.gpsimd, t1, t1, t2, OP.add)
        tt(nc.gpsimd, t1, t1, t3, OP.add)
        nc.vector.scalar_tensor_tensor(out=p2[:], in0=t1[:], scalar=2.0, in1=t0[:], op0=OP.mult, op1=OP.add)
        p = small("p")
        act(p, p2, AF.Sqrt, scale=1.0 / 6.0)
        # newton-refine p (ACT sqrt table has a loose precision budget):
        # p <- 0.5 * (p + (p2/6) / p)
        invp0 = small("invp0")
        nc.vector.reciprocal(invp0[:], p[:])
        nc.vector.tensor_scalar(out=t0[:], in0=p2[:], scalar1=1.0 / 6.0, scalar2=None, op0=OP.mult)
        tt(nc.vector, t0, t0, invp0, OP.mult)
        tt(nc.vector, t0, t0, p, OP.add)
        nc.vector.tensor_scalar(out=p[:], in0=t0[:], scalar1=0.5, scalar2=None, op0=OP.mult)

        invp = small("invp")
        nc.vector.reciprocal(invp[:], p[:])

        # normalized traceless C = (A - qI)/p ; r = det(C)/2 clamped to [-1,1]
        c11 = small("c11"); c22 = small("c22"); c33 = small("c33")
        c12 = small("c12"); c13 = small("c13"); c23 = small("c23")
        tt(nc.vector, c11, b11, invp, OP.mult)
        tt(nc.vector, c22, b22, invp, OP.mult)
        tt(nc.vector, c33, b33, invp, OP.mult)
        tt(nc.gpsimd, c12, axy, invp, OP.mult)
        tt(nc.gpsimd, c13, axz, invp, OP.mult)
        tt(nc.gpsimd, c23, ayz, invp, OP.mult)

        r = small("r")
        tt(nc.vector, t0, c22, c33, OP.mult)
        act(t1, c23, AF.Square)
        tt(nc.vector, t0, t0, t1, OP.subtract)      # m1
        tt(nc.vector, t0, t0, c11, OP.mult)         # c11*m1
        tt(nc.gpsimd, t1, c12, c33, OP.mult)
        tt(nc.gpsimd, t2, c23, c13, OP.mult)
        tt(nc.gpsimd, t1, t1, t2, OP.subtract)      # m2
        tt(nc.gpsimd, t1, t1, c12, OP.mult)         # c12*m2
        tt(nc.vector, t2, c12, c23, OP.mult)
        tt(nc.vector, t3, c22, c13, OP.mult)
        tt(nc.vector, t2, t2, t3, OP.subtract)      # m3
        tt(nc.vector, t2, t2, c13, OP.mult)         # c13*m3
        tt(nc.vector, t0, t0, t1, OP.subtract)
        tt(nc.vector, t0, t0, t2, OP.add)           # det
        nc.vector.tensor_scalar(out=r[:], in0=t0[:], scalar1=0.5, scalar2=1.0, op0=OP.mult, op1=OP.min)
        nc.vector.tensor_scalar(out=r[:], in0=r[:], scalar1=-1.0, scalar2=None, op0=OP.max)

        # theta = acos(r) = 4*arctan( sqrt((1-r)/2) / (1 + sqrt((1+r)/2)) )
        # (quarter-angle form keeps the arctan argument in [0, 1] — the ACT
        # arctan table only covers [-pi/2, pi/2])
        at4 = small("at4")
        nc.vector.tensor_scalar(out=t0[:], in0=r[:], scalar1=-0.5, scalar2=0.5, op0=OP.mult, op1=OP.add)
        nc.vector.tensor_scalar(out=t1[:], in0=r[:], scalar1=0.5, scalar2=0.5, op0=OP.mult, op1=OP.add)
        sa = small("sa"); sb = small("sb")
        act(sa, t0, AF.Sqrt)
        act(sb, t1, AF.Sqrt)
        # newton-refine both sqrts (guarded): s <- 0.5*(s + v/s)
        nc.vector.tensor_scalar(out=sa[:], in0=sa[:], scalar1=1e-30, scalar2=None, op0=OP.max)
        nc.vector.reciprocal(t2[:], sa[:])
        tt(nc.vector, t3, t0, t2, OP.mult)
        tt(nc.vector, sa, sa, t3, OP.add)
        nc.vector.tensor_scalar(out=sa[:], in0=sa[:], scalar1=0.5, scalar2=None, op0=OP.mult)
        nc.vector.tensor_scalar(out=sb[:], in0=sb[:], scalar1=1e-30, scalar2=None, op0=OP.max)
        nc.vector.reciprocal(t2[:], sb[:])
        tt(nc.vector, t3, t1, t2, OP.mult)
        tt(nc.vector, sb, sb, t3, OP.add)
        nc.vector.tensor_scalar(out=sb[:], in0=sb[:], scalar1=0.5, scalar2=1.0, op0=OP.mult, op1=OP.add)  # 1 + sqrt((1+r)/2)
        nc.vector.reciprocal(t2[:], sb[:])
        tt(nc.vector, t3, sa, t2, OP.mult)          # tan(theta/4) in [0, 1]
        act(at4, t3, AF.Arctan)

        # cos(theta/3) = sin(pi/2 - (4/3)*at4);  sin(pi/6 + theta/3) = sin(pi/6 + (4/3)*at4)
        cmax = small("cmax"); smin = small("smin")
        act(cmax, at4, AF.Sin, bias=bias_pi2[:, 0:1], scale=-4.0 / 3.0)
        act(smin, at4, AF.Sin, bias=bias_pi6[:, 0:1], scale=4.0 / 3.0)

        # eigenvalues: w3 = q + 2p*cmax (max), w1 = q - 2p*smin (min), w2 = 3q - w3 - w1
        w3 = small("w3"); w2 = small("w2")
        tt(nc.vector, t0, p, cmax, OP.mult)
        tt(nc.vector, t0, t0, t0, OP.add)  # 2*p*cmax
        tt(nc.vector, w3, q, t0, OP.add)
        tt(nc.gpsimd, t1, p, smin, OP.mult)
        tt(nc.gpsimd, t1, t1, t1, OP.add)
        tt(nc.gpsimd, t1, q, t1, OP.subtract)       # w1
        nc.vector.tensor_scalar(out=t2[:], in0=q[:], scalar1=3.0, scalar2=None, op0=OP.mult)
        tt(nc.vector, t2, t2, w3, OP.subtract)
        tt(nc.vector, w2, t2, t1, OP.subtract)

        invw3 = small("invw3")
        nc.vector.reciprocal(invw3[:], w3[:])
        dirwt = small("dirwt")
        tt(nc.vector, t0, w2, invw3, OP.mult)
        nc.vector.tensor_scalar(out=dirwt[:], in0=t0[:], scalar1=-1.0, scalar2=1.0, op0=OP.mult, op1=OP.add)

        # ---- eigenvector for w3: cross products of rows of (A - w3 I) ----
        d1 = small("d1"); d2 = small("d2"); d3 = small("d3")
        tt(nc.vector, d1, axx, w3, OP.subtract)
        tt(nc.vector, d2, ayy, w3, OP.subtract)
        tt(nc.vector, d3, azz, w3, OP.subtract)

        u1 = small("u1"); u2 = small("u2"); u3 = small("u3")
        tt(nc.vector, u1, axy, ayz, OP.mult)
        tt(nc.gpsimd, t0, d2, axz, OP.mult)
        tt(nc.vector, u1, u1, t0, OP.subtract)
        tt(nc.vector, u2, axy, axz, OP.mult)
        tt(nc.gpsimd, t1, d1, ayz, OP.mult)
        tt(nc.vector, u2, u2, t1, OP.subtract)
        tt(nc.vector, u3, d1, d2, OP.mult)
        act(t2, axy, AF.Square)
        tt(nc.vector, u3, u3, t2, OP.subtract)

        v1 = small("v1"); v2 = small("v2"); v3_ = small("v3_")
        tt(nc.vector, v1, axy, d3, OP.mult)
        tt(nc.gpsimd, t0, axz, ayz, OP.mult)
        tt(nc.vector, v1, v1, t0, OP.subtract)
        act(v2, axz, AF.Square)
        tt(nc.gpsimd, t1, d1, d3, OP.mult)
        tt(nc.vector, v2, v2, t1, OP.subtract)
        tt(nc.vector, v3_, d1, ayz, OP.mult)
        tt(nc.gpsimd, t2, axy, axz, OP.mult)
        tt(nc.vector, v3_, v3_, t2, OP.subtract)

        k1 = small("k1"); k2 = small("k2"); k3 = small("k3")
        tt(nc.vector, k1, d2, d3, OP.mult)
        act(t0, ayz, AF.Square)
        tt(nc.vector, k1, k1, t0, OP.subtract)
        tt(nc.vector, k2, ayz, axz, OP.mult)
        tt(nc.gpsimd, t1, axy, d3, OP.mult)
        tt(nc.vector, k2, k2, t1, OP.subtract)
        tt(nc.vector, k3, axy, ayz, OP.mult)
        tt(nc.gpsimd, t2, d2, axz, OP.mult)
        tt(nc.vector, k3, k3, t2, OP.subtract)

        # squared norms
        nu = small("nu"); nv = small("nv"); nk = small("nk")
        for (n_, e1, e2, e3) in ((nu, u1, u2, u3), (nv, v1, v2, v3_), (nk, k1, k2, k3)):
            act(t0, e1, AF.Square)
            act(t1, e2, AF.Square)
            act(t2, e3, AF.Square)
            tt(nc.vector, t0, t0, t1, OP.add)
            tt(nc.vector, n_, t0, t2, OP.add)

        # pick the largest-norm candidate (select mask must be integer dtype)
        m = pool.tile([P, NSEG], mybir.dt.uint8, tag="s_mask", name="m")
        e1 = small("e1"); e2 = small("e2"); e3 = small("e3"); ne = small("ne")
        tt(nc.vector, m, nv, nu, OP.is_gt)
        nc.vector.select(e1[:], m[:], v1[:], u1[:])
        nc.vector.select(e2[:], m[:], v2[:], u2[:])
        nc.vector.select(e3[:], m[:], v3_[:], u3[:])
        nc.vector.select(ne[:], m[:], nv[:], nu[:])
        tt(nc.vector, m, nk, ne, OP.is_gt)
        nc.vector.select(e1[:], m[:], k1[:], e1[:])
        nc.vector.select(e2[:], m[:], k2[:], e2[:])
        nc.vector.select(e3[:], m[:], k3[:], e3[:])
        nc.vector.select(ne[:], m[:], nk[:], ne[:])

        # normalize: v0 = e / sqrt(ne)   (sqrt + newton refine)
        act(t0, ne, AF.Sqrt)
        nc.vector.tensor_scalar(out=t0[:], in0=t0[:], scalar1=1e-30, scalar2=None, op0=OP.max)
        nc.vector.reciprocal(t1[:], t0[:])
        tt(nc.vector, t2, ne, t1, OP.mult)
        tt(nc.vector, t0, t0, t2, OP.add)
        nc.vector.tensor_scalar(out=t0[:], in0=t0[:], scalar1=0.5, scalar2=1e-30, op0=OP.mult, op1=OP.max)
        invn = small("invn")
        nc.vector.reciprocal(invn[:], t0[:])
        v0x = small("v0x"); v0y = small("v0y"); v0z = small("v0z")
        tt(nc.vector, v0x, e1, invn, OP.mult)
        tt(nc.vector, v0y, e2, invn, OP.mult)
        tt(nc.vector, v0z, e3, invn, OP.mult)

        # ---------------- phase 2: sign criterion ----------------
        # center in place: X <- X - cx (broadcast)
        nc.gpsimd.tensor_tensor(X[:], X[:], _bcast(cx), OP.subtract)
        nc.gpsimd.tensor_tensor(Y[:], Y[:], _bcast(cy), OP.subtract)
        nc.gpsimd.tensor_tensor(Z[:], Z[:], _bcast(cz), OP.subtract)

        # x0 = Xc*v0x + Yc*v0y + Zc*v0z
        x0 = plane("x0")
        w0 = plane("w0")
        nc.vector.tensor_tensor(x0[:], X[:], _bcast(v0x), OP.mult)
        nc.vector.tensor_tensor(w0[:], Y[:], _bcast(v0y), OP.mult)
        nc.vector.tensor_tensor(x0[:], x0[:], w0[:], OP.add)
        nc.vector.tensor_tensor(w0[:], Z[:], _bcast(v0z), OP.mult)
        nc.vector.tensor_tensor(x0[:], x0[:], w0[:], OP.add)

        # n2 = (Xc^2+Yc^2+Zc^2) - x0^2, clamped at 0; np0 = sqrt(n2)
        r2 = plane("r2")
        s1 = plane("s1")
        s2 = plane("s2")
        nc.scalar.activation(r2[:], X[:], AF.Square)
        nc.scalar.activation(s1[:], Y[:], AF.Square)
        nc.scalar.activation(s2[:], Z[:], AF.Square)
        nc.vector.tensor_tensor(r2[:], r2[:], s1[:], OP.add)
        nc.gpsimd.tensor_tensor(r2[:], r2[:], s2[:], OP.add)
        nc.scalar.activation(s1[:], x0[:], AF.Square)
        nc.vector.tensor_tensor(r2[:], r2[:], s1[:], OP.subtract)
        nc.vector.tensor_scalar(out=r2[:], in0=r2[:], scalar1=0.0, scalar2=None, op0=OP.max)
        np0 = plane("np0")
        nc.scalar.activation(np0[:], r2[:], AF.Sqrt)
        # pr = x0 * np0 ; sc = sum_s pr
        nc.vector.tensor_tensor(np0[:], np0[:], x0[:], OP.mult)
        sc = small("sc")
        nc.vector.tensor_reduce(sc[:], np0[:], axis=AX.X, op=OP.add)

        # ---------------- finalize: feats [P, NSEG, 16] ----------------
        feats = pool.tile([P, NSEG, 16], F32, tag="feats")
        # fac = dirwt * (sc < 0 ? -1 : 1)
        fac = small("fac")
        nc.vector.tensor_scalar(out=t0[:], in0=sc[:], scalar1=0.0, scalar2=-2.0, op0=OP.is_lt, op1=OP.mult)
        nc.vector.tensor_scalar(out=t0[:], in0=t0[:], scalar1=1.0, scalar2=None, op0=OP.add)
        tt(nc.vector, fac, t0, dirwt, OP.mult)

        nc.vector.tensor_copy(feats[:, :, 0], cx[:])
        nc.vector.tensor_copy(feats[:, :, 1], cy[:])
        nc.vector.tensor_copy(feats[:, :, 2], cz[:])
        # B = A / w3  (9 entries, B is symmetric)
        tt(nc.vector, t0, axx, invw3, OP.mult)
        nc.vector.tensor_copy(feats[:, :, 3], t0[:])
        tt(nc.vector, t0, axy, invw3, OP.mult)
        nc.vector.tensor_copy(feats[:, :, 4], t0[:])
        nc.vector.tensor_copy(feats[:, :, 6], t0[:])
        tt(nc.vector, t0, axz, invw3, OP.mult)
        nc.vector.tensor_copy(feats[:, :, 5], t0[:])
        nc.vector.tensor_copy(feats[:, :, 9], t0[:])
        tt(nc.vector, t0, ayy, invw3, OP.mult)
        nc.vector.tensor_copy(feats[:, :, 7], t0[:])
        tt(nc.vector, t0, ayz, invw3, OP.mult)
        nc.vector.tensor_copy(feats[:, :, 8], t0[:])
        nc.vector.tensor_copy(feats[:, :, 10], t0[:])
        tt(nc.vector, t0, azz, invw3, OP.mult)
        nc.vector.tensor_copy(feats[:, :, 11], t0[:])
        tt(nc.vector, t0, v0x, fac, OP.mult)
        nc.vector.tensor_copy(feats[:, :, 12], t0[:])
        tt(nc.vector, t0, v0y, fac, OP.mult)
        nc.vector.tensor_copy(feats[:, :, 13], t0[:])
        tt(nc.vector, t0, v0z, fac, OP.mult)
        nc.vector.tensor_copy(feats[:, :, 14], t0[:])
        size_t = small("size_t")
        nc.gpsimd.memset(size_t[:], float(V))
        nc.vector.tensor_copy(feats[:, :, 15], size_t[:])

        nc.sync.dma_start(feats_d.rearrange("(p s) k -> p s k", p=P), feats[:])

    if not nc.is_finalized():
        nc.finalize()
    return nc


def kernel(data: np.ndarray, clusts: np.ndarray) -> np.ndarray:
    data = np.ascontiguousarray(np.asarray(data, dtype=np.float32))
    clusts_np = np.asarray(clusts)
    C, S = clusts_np.shape
    assert (C, S) == (N_CLUSTS, CLUST_SIZE), (C, S)

    # host-side pre-partition: gather each cluster's voxel coordinates
    vox = data[:, 1:4]
    g = vox[clusts_np.reshape(-1).astype(np.int64)].reshape(C, S, 3)
    xs = np.ascontiguousarray(g[:, :, 0])
    ys = np.ascontiguousarray(g[:, :, 1])
    zs = np.ascontiguousarray(g[:, :, 2])

    if "nc" not in _CACHED:
        _CACHED["nc"] = build_nc()
    nc = _CACHED["nc"]

    def tmajor(a):
        # voxel-major plane whose column j*128+m is cluster m*32+j, so the
        # PE column-sum (chunk j -> psum partition m) lands exactly at the
        # kernel's cluster slot (partition m, segment j)
        return np.ascontiguousarray(
            a.reshape(P, NSEG, V).transpose(2, 1, 0).reshape(V, C_LOC))

    in_maps = []
    for c in range(N_CORES):
        sl = slice(c * C_LOC, (c + 1) * C_LOC)
        in_maps.append({
            "x": xs[sl], "y": ys[sl], "z": zs[sl],
            "xt": tmajor(xs[sl]),
            "yt": tmajor(ys[sl]),
            "zt": tmajor(zs[sl]),
        })

    kw = {}
    if PROFILE:
        kw = dict(trace=True)
    res = run_bass_kernel_spmd(nc, in_maps, list(range(N_CORES)), **kw)
    if PROFILE:
        global LAST_RESULT
        LAST_RESULT = res
    out = np.concatenate([res.results[c]["feats"] for c in range(N_CORES)], axis=0)
    return out.astype(np.float32)



# revision 12
# speedup vs baseline: 1.5498x; 1.5498x over previous
"""Trainium2 Bass kernel: per-cluster PCA geometry features (segment reduce).

Problem: data [4194304, 6] f32, clusts [32768, 128] int — per cluster of 128
voxels compute: center (mean of xyz), normalized covariance B = A/lmax,
principal axis v0 scaled by dirwt = 1 - lmid/lmax with a sign fix, size.

v2 design (cost-model driven):
- 32768 clusters sharded over 8 cores (4096 each); cluster c = p*32 + s
  (partition p, segment s). All plane data in bf16.
- Dual layout per coordinate: voxel-major [V, C_LOC] for PE column sums
  (phase-1 moments), cluster-major *segment-minor* [P, V, S] so per-cluster
  scalar broadcasts keep a packed (stride-1) last dim -> DVE 2x mode.
- Phase 1: squares on ACT, cross products on GPSIMD, sums via 288 tiny
  PE matmuls into one PSUM tile.
- Eigensolve: batched small-tile ops on [128, K, 32] tiles (trig method),
  eigenvector via row-cross-products on a doubled-rows tile so the three
  candidate cross products are affine-sliced batched ops.
- Phase 2 (per segment-half, pipelined): xc = X - c, r2 = |xc|^2 via ACT
  squares + GPSIMD adds, x0' = xc . e (unnormalized eigvec, avoids a sqrt
  on the critical path), h = r2 - x0'^2/|e|^2, np0 = ACT sqrt of clamped h,
  sign criterion sc = sum_v x0'*np0 via packed tree-reduction.
"""
import numpy as np
from contextlib import ExitStack

import concourse.bass as bass
import concourse.bacc as bacc
import concourse.tile as tile
from concourse import mybir
from concourse.bass_utils import run_bass_kernel_spmd

N_CLUSTS = 32768
CLUST_SIZE = 128
N_CORES = 8
C_LOC = N_CLUSTS // N_CORES   # 4096 clusters per core
P = 128                       # SBUF partitions
NSEG = C_LOC // P             # 32 clusters (segments) per partition
V = CLUST_SIZE                # 128 voxels per cluster
HALF = NSEG // 2

F32 = mybir.dt.float32
BF16 = mybir.dt.bfloat16
AF = mybir.ActivationFunctionType
OP = mybir.AluOpType
AX = mybir.AxisListType

_CACHED = {}
PROFILE = False          # set by test.py to capture an NTFF trace
LAST_RESULT = None       # BassKernelResults of the last run when PROFILE
INPUT_SHAPES = {
    "x": (P, V * NSEG), "y": (P, V * NSEG), "z": (P, V * NSEG),
    "xt": (V, C_LOC), "yt": (V, C_LOC), "zt": (V, C_LOC),
}
INPUT_DTYPE = "bfloat16"


def build_nc():
    nc = bacc.Bacc()
    # cluster-major segment-minor: row p holds [v, s] (v outer, s inner)
    x_d = nc.dram_tensor("x", [P, V * NSEG], BF16, kind="ExternalInput").ap()
    y_d = nc.dram_tensor("y", [P, V * NSEG], BF16, kind="ExternalInput").ap()
    z_d = nc.dram_tensor("z", [P, V * NSEG], BF16, kind="ExternalInput").ap()
    # voxel-major: column s*128+p holds cluster p*32+s
    xt_d = nc.dram_tensor("xt", [V, C_LOC], BF16, kind="ExternalInput").ap()
    yt_d = nc.dram_tensor("yt", [V, C_LOC], BF16, kind="ExternalInput").ap()
    zt_d = nc.dram_tensor("zt", [V, C_LOC], BF16, kind="ExternalInput").ap()
    feats_d = nc.dram_tensor("feats", [C_LOC, 16], F32, kind="ExternalOutput").ap()

    with tile.TileContext(nc) as tc, ExitStack() as ctx:
        pool = ctx.enter_context(tc.tile_pool(name="main", bufs=1))
        sp = ctx.enter_context(tc.tile_pool(name="scratch", bufs=3))
        pp = ctx.enter_context(tc.tile_pool(name="psum", bufs=1, space="PSUM"))

        ones = pool.tile([P, 1], BF16, tag="ones")
        nc.gpsimd.memset(ones[:], 1.0)
        bias_pi2 = pool.tile([P, 1], F32, tag="bias_pi2")
        bias_pi6 = pool.tile([P, 1], F32, tag="bias_pi6")
        nc.gpsimd.memset(bias_pi2[:], 1.5707963267948966)
        nc.gpsimd.memset(bias_pi6[:], 0.5235987755982988)

        # ---------------- input DMAs ----------------
        Xt = pool.tile([P, C_LOC], BF16, tag="Xt")
        Yt = pool.tile([P, C_LOC], BF16, tag="Yt")
        Zt = pool.tile([P, C_LOC], BF16, tag="Zt")
        X = pool.tile([P, V, NSEG], BF16, tag="X")
        Y = pool.tile([P, V, NSEG], BF16, tag="Y")
        Z = pool.tile([P, V, NSEG], BF16, tag="Z")
        nc.sync.dma_start(Xt[:], xt_d)
        nc.sync.dma_start(Yt[:], yt_d)
        nc.sync.dma_start(Zt[:], zt_d)
        nc.sync.dma_start(X[:], x_d.rearrange("p (v s) -> p v s", s=NSEG))
        nc.sync.dma_start(Y[:], y_d.rearrange("p (v s) -> p v s", s=NSEG))
        nc.sync.dma_start(Z[:], z_d.rearrange("p (v s) -> p v s", s=NSEG))

        # ---------------- phase 1: moments via PE column sums ----------------
        psum = pp.tile([P, 9, NSEG], F32, tag="psums")

        def pe_colsums(plane_t, k):
            for j in range(NSEG):
                nc.tensor.matmul(
                    out=psum[:, k, j : j + 1],
                    lhsT=plane_t[:, j * P : (j + 1) * P],
                    rhs=ones[:, 0:1],
                    start=True,
                    stop=True,
                )

        pe_colsums(Xt, 0)
        pe_colsums(Yt, 1)
        pe_colsums(Zt, 2)
        sq1 = sp.tile([P, C_LOC], BF16, tag="vplane", name="sq1")
        nc.scalar.activation(sq1[:], Xt[:], AF.Square)
        pe_colsums(sq1, 3)
        sq2 = sp.tile([P, C_LOC], BF16, tag="vplane", name="sq2")
        nc.scalar.activation(sq2[:], Yt[:], AF.Square)
        pe_colsums(sq2, 4)
        sq3 = sp.tile([P, C_LOC], BF16, tag="vplane", name="sq3")
        nc.scalar.activation(sq3[:], Zt[:], AF.Square)
        pe_colsums(sq3, 5)
        pr1 = sp.tile([P, C_LOC], BF16, tag="vplane", name="pr1")
        nc.gpsimd.tensor_tensor(pr1[:], Xt[:], Yt[:], OP.mult)
        pe_colsums(pr1, 6)
        pr2 = sp.tile([P, C_LOC], BF16, tag="vplane", name="pr2")
        nc.gpsimd.tensor_tensor(pr2[:], Xt[:], Zt[:], OP.mult)
        pe_colsums(pr2, 7)
        pr3 = sp.tile([P, C_LOC], BF16, tag="vplane", name="pr3")
        nc.gpsimd.tensor_tensor(pr3[:], Yt[:], Zt[:], OP.mult)
        pe_colsums(pr3, 8)

        # moments: rows 0-2 = S (first), rows 3-8 = [Mxx Myy Mzz Mxy Mxz Myz]
        mom = pool.tile([P, 9, NSEG], F32, tag="mom")
        nc.vector.tensor_copy(mom[:], psum[:])

        inv_s = 1.0 / V
        c3 = pool.tile([P, 3, NSEG], F32, tag="c3")
        nc.vector.tensor_scalar(out=c3[:], in0=mom[:, 0:3], scalar1=inv_s,
                                scalar2=None, op0=OP.mult)
        cb = pool.tile([P, 3, NSEG], BF16, tag="cb")
        nc.vector.tensor_copy(cb[:], c3[:])

        def bcast_v(t, lo, hi, sl, n=1):
            # [P, K, NSEG] rows lo:hi, segments sl -> [P, n, V, W] broadcast
            w = sl.stop - sl.start
            if n == 1:
                return t[:, lo:hi, sl].broadcast_to([P, V, w]) if hi - lo == 1 \
                    else None
            return None

        # a = M - S*c ; amat rows: [axx, ayy, azz, axy, axz, ayz]
        amat = pool.tile([P, 6, NSEG], F32, tag="amat")
        t6 = pool.tile([P, 6, NSEG], F32, tag="t6")
        nc.vector.tensor_tensor(t6[:, 0:3], mom[:, 0:3], c3[:], OP.mult)
        nc.vector.tensor_tensor(
            t6[:, 3:5], mom[:, 0:1].broadcast_to([P, 2, NSEG]),
            c3[:, 1:3], OP.mult)
        nc.vector.tensor_tensor(t6[:, 5:6], mom[:, 1:2], c3[:, 2:3], OP.mult)
        nc.vector.tensor_tensor(amat[:], mom[:, 3:9], t6[:], OP.subtract)

        # ---- phase 2 prep (independent of eigen): xc, squares, r2 ---------
        planes = [X, Y, Z]
        r2h = [pool.tile([P, V, HALF], BF16, tag=f"r2h{h}", name=f"r2h{h}")
               for h in range(2)]
        x0h = [pool.tile([P, V, HALF], BF16, tag=f"x0h{h}", name=f"x0h{h}")
               for h in range(2)]
        sqh = [sp.tile([P, V, HALF], BF16, tag="sqh", name=f"sqh{h}")
               for h in range(2)]

        def cbc(t, row, h, w=HALF):
            # [P, K, NSEG] row -> [P, V, w] broadcast over voxels
            return t[:, row:row + 1, h * HALF:h * HALF + w].broadcast_to(
                [P, V, w])

        for h in range(2):
            sl = slice(h * HALF, (h + 1) * HALF)
            for i, pl in enumerate(planes):
                nc.vector.tensor_tensor(pl[:, :, sl], pl[:, :, sl],
                                        cbc(cb, i, h), OP.subtract)
            nc.scalar.activation(r2h[h][:], X[:, :, sl], AF.Square)
            nc.scalar.activation(sqh[h][:], Y[:, :, sl], AF.Square)
            nc.gpsimd.tensor_tensor(r2h[h][:], r2h[h][:], sqh[h][:], OP.add)
            nc.scalar.activation(sqh[h][:], Z[:, :, sl], AF.Square)
            nc.gpsimd.tensor_tensor(r2h[h][:], r2h[h][:], sqh[h][:], OP.add)

        # ---------------- eigensolve (batched [P, K, NSEG] f32) -------------
        q = pool.tile([P, 1, NSEG], F32, tag="q")
        t1a = pool.tile([P, 1, NSEG], F32, tag="t1a")
        t1b = pool.tile([P, 1, NSEG], F32, tag="t1b")
        nc.vector.tensor_tensor(t1a[:], amat[:, 0:1], amat[:, 1:2], OP.add)
        nc.vector.tensor_tensor(t1a[:], t1a[:], amat[:, 2:3], OP.add)
        nc.vector.tensor_scalar(out=q[:], in0=t1a[:], scalar1=1.0 / 3.0,
                                scalar2=None, op0=OP.mult)
        # ba rows: [b11, b22, b33, axy, axz, ayz]  (traceless part)
        ba = pool.tile([P, 6, NSEG], F32, tag="ba")
        nc.vector.tensor_tensor(
            ba[:, 0:3], amat[:, 0:3], q[:].broadcast_to([P, 3, NSEG]),
            OP.subtract)
        nc.vector.tensor_copy(ba[:, 3:6], amat[:, 3:6])

        # p2 = sum(bd^2) + 2*sum(off^2) ; p = sqrt(p2/6); invp = 1/p
        sq6 = pool.tile([P, 6, NSEG], F32, tag="sq6")
        nc.vector.tensor_tensor(sq6[:], ba[:], ba[:], OP.mult)
        w3t = pool.tile([P, 3, NSEG], F32, tag="w3t")
        nc.vector.scalar_tensor_tensor(out=w3t[:], in0=sq6[:, 3:6], scalar=2.0,
                                       in1=sq6[:, 0:3], op0=OP.mult, op1=OP.add)
        nc.vector.tensor_tensor(t1a[:], w3t[:, 0:1], w3t[:, 1:2], OP.add)
        nc.vector.tensor_tensor(t1a[:], t1a[:], w3t[:, 2:3], OP.add)
        p_t = pool.tile([P, 1, NSEG], F32, tag="p_t")
        nc.scalar.activation(p_t[:], t1a[:], AF.Sqrt, scale=1.0 / 6.0)
        invp = pool.tile([P, 1, NSEG], F32, tag="invp")
        nc.vector.reciprocal(invp[:], p_t[:])

        cmat = pool.tile([P, 6, NSEG], F32, tag="cmat")
        nc.vector.tensor_tensor(
            cmat[:], ba[:], invp[:].broadcast_to([P, 6, NSEG]), OP.mult)

        # r = det(cmat)/2 clamped to [-1, 1]
        # minors vs row1: m1 = c22*c33 - c23^2 ; m2 = c12*c33 - c23*c13 ;
        # m3 = c12*c23 - c22*c13 ; det = c11*m1 - c12*m2 + c13*m3
        r_t = pool.tile([P, 1, NSEG], F32, tag="r_t")
        det_a = pool.tile([P, 3, NSEG], F32, tag="det_a")
        det_b = pool.tile([P, 3, NSEG], F32, tag="det_b")
        nc.gpsimd.tensor_tensor(det_a[:, 0:1], cmat[:, 1:2], cmat[:, 2:3],
                                OP.mult)
        nc.gpsimd.tensor_tensor(
            det_a[:, 1:3], cmat[:, 3:4].broadcast_to([P, 2, NSEG]),
            cmat[:, 2:6:3], OP.mult)
        nc.gpsimd.tensor_tensor(det_b[:, 0:1], cmat[:, 5:6], cmat[:, 5:6],
                                OP.mult)
        nc.gpsimd.tensor_tensor(det_b[:, 1:2], cmat[:, 5:6], cmat[:, 4:5],
                                OP.mult)
        nc.gpsimd.tensor_tensor(det_b[:, 2:3], cmat[:, 1:2], cmat[:, 4:5],
                                OP.mult)
        nc.gpsimd.tensor_tensor(det_a[:], det_a[:], det_b[:], OP.subtract)
        nc.gpsimd.tensor_tensor(det_b[:, 0:1], cmat[:, 0:1], det_a[:, 0:1],
                                OP.mult)
        nc.gpsimd.tensor_tensor(det_b[:, 1:3], cmat[:, 3:5], det_a[:, 1:3],
                                OP.mult)
        nc.gpsimd.tensor_tensor(t1b[:], det_b[:, 0:1], det_b[:, 1:2],
                                OP.subtract)
        nc.gpsimd.tensor_tensor(t1b[:], t1b[:], det_b[:, 2:3], OP.add)
        nc.vector.tensor_scalar(out=r_t[:], in0=t1b[:], scalar1=0.5,
                                scalar2=1.0, op0=OP.mult, op1=OP.min)
        nc.vector.tensor_scalar(out=r_t[:], in0=r_t[:], scalar1=-1.0,
                                scalar2=None, op0=OP.max)

        # theta/4 form: at4 = arctan(sqrt((1-r)/2) / (1 + sqrt((1+r)/2)))
        sa = pool.tile([P, 1, NSEG], F32, tag="sa")
        sb = pool.tile([P, 1, NSEG], F32, tag="sb")
        nc.vector.tensor_scalar(out=t1a[:], in0=r_t[:], scalar1=-0.5,
                                scalar2=0.5, op0=OP.mult, op1=OP.add)
        nc.scalar.activation(sa[:], t1a[:], AF.Sqrt)
        nc.vector.tensor_scalar(out=t1b[:], in0=r_t[:], scalar1=0.5,
                                scalar2=0.5, op0=OP.mult, op1=OP.add)
        nc.scalar.activation(sb[:], t1b[:], AF.Sqrt)
        nc.vector.tensor_scalar(out=sb[:], in0=sb[:], scalar1=1.0,
                                scalar2=None, op0=OP.add)
        nc.vector.reciprocal(t1a[:], sb[:])
        nc.vector.tensor_tensor(t1a[:], sa[:], t1a[:], OP.mult)
        at4 = pool.tile([P, 1, NSEG], F32, tag="at4")
        nc.scalar.activation(at4[:], t1a[:], AF.Arctan)

        cmax = pool.tile([P, 1, NSEG], F32, tag="cmax")
        smin = pool.tile([P, 1, NSEG], F32, tag="smin")
        nc.scalar.activation(cmax[:], at4[:], AF.Sin, bias=bias_pi2[:, 0:1],
                             scale=-4.0 / 3.0)
        nc.scalar.activation(smin[:], at4[:], AF.Sin, bias=bias_pi6[:, 0:1],
                             scale=4.0 / 3.0)

        # w3 = q + 2 p cmax ; w1 = q - 2 p smin ; w2 = 3q - w3 - w1
        w3 = pool.tile([P, 1, NSEG], F32, tag="w3")
        w2 = pool.tile([P, 1, NSEG], F32, tag="w2")
        nc.vector.tensor_tensor(t1a[:], p_t[:], cmax[:], OP.mult)
        nc.vector.scalar_tensor_tensor(out=w3[:], in0=t1a[:], scalar=2.0,
                                       in1=q[:], op0=OP.mult, op1=OP.add)
        nc.vector.tensor_tensor(t1b[:], p_t[:], smin[:], OP.mult)
        nc.vector.scalar_tensor_tensor(out=t1b[:], in0=t1b[:], scalar=-2.0,
                                       in1=q[:], op0=OP.mult, op1=OP.add)
        nc.vector.scalar_tensor_tensor(out=w2[:], in0=q[:], scalar=3.0,
                                       in1=w3[:], op0=OP.mult, op1=OP.subtract)
        nc.vector.tensor_tensor(w2[:], w2[:], t1b[:], OP.subtract)

        invw3 = pool.tile([P, 1, NSEG], F32, tag="invw3")
        nc.vector.reciprocal(invw3[:], w3[:])
        dirwt = pool.tile([P, 1, NSEG], F32, tag="dirwt")
        nc.vector.tensor_tensor(dirwt[:], w2[:], invw3[:], OP.mult)
        nc.vector.tensor_scalar(out=dirwt[:], in0=dirwt[:], scalar1=-1.0,
                                scalar2=1.0, op0=OP.mult, op1=OP.add)

        # ---- eigenvector: batched row cross products -------------------
        # rows of (A - w3 I): r1 = (d1, axy, axz), r2 = (axy, d2, ayz),
        # r3 = (axz, ayz, d3); D = per-vector doubled rows.
        rows = pool.tile([P, 9, NSEG], F32, tag="rows")
        nc.vector.tensor_tensor(
            rows[:, 0:9:4], amat[:, 0:3],
            w3[:].broadcast_to([P, 3, NSEG]), OP.subtract)
        nc.vector.tensor_copy(rows[:, 1:2], amat[:, 3:4])   # axy
        nc.vector.tensor_copy(rows[:, 2:3], amat[:, 4:5])   # axz
        nc.vector.tensor_copy(rows[:, 3:4], amat[:, 3:4])   # axy
        nc.vector.tensor_copy(rows[:, 5:6], amat[:, 5:6])   # ayz
        nc.vector.tensor_copy(rows[:, 6:7], amat[:, 4:5])   # axz
        nc.vector.tensor_copy(rows[:, 7:8], amat[:, 5:6])   # ayz
        D = pool.tile([P, 3, 2, 3, NSEG], F32, tag="D")
        rows_g = rows[:].rearrange("p (g c) s -> p g c s", g=3)
        nc.vector.tensor_copy(D[:, :, 0], rows_g)
        nc.vector.tensor_copy(D[:, :, 1], rows_g)
        Dg = D[:].rearrange("p g r c s -> p g (r c) s")  # [P, 3, 6, NSEG]

        # VN rows: cand_a(0:3), na(3), cand_b(4:7), nb(7), cand_c(8:11), nc(11)
        VN = pool.tile([P, 3, 4, NSEG], F32, tag="VN")
        ta = pool.tile([P, 2, 3, NSEG], F32, tag="ta")
        tb = pool.tile([P, 2, 3, NSEG], F32, tag="tb")
        r1s1 = Dg[:, 0:1, 1:4].broadcast_to([P, 2, 3, NSEG])
        r1s2 = Dg[:, 0:1, 2:5].broadcast_to([P, 2, 3, NSEG])
        nc.vector.tensor_tensor(ta[:], r1s1, Dg[:, 1:3, 2:5], OP.mult)
        nc.vector.tensor_tensor(tb[:], r1s2, Dg[:, 1:3, 1:4], OP.mult)
        nc.vector.tensor_tensor(ta[:], ta[:], tb[:], OP.subtract)
        nc.vector.tensor_copy(VN[:, 0:2, 0:3], ta[:])
        nc.vector.tensor_tensor(tb[:, 0], Dg[:, 1, 1:4], Dg[:, 2, 2:5], OP.mult)
        nc.vector.tensor_tensor(tb[:, 1], Dg[:, 1, 2:5], Dg[:, 2, 1:4], OP.mult)
        nc.vector.tensor_tensor(VN[:, 2, 0:3], tb[:, 0], tb[:, 1], OP.subtract)

        # norms into VN[:, :, 3]
        sq9 = pool.tile([P, 3, 3, NSEG], F32, tag="sq9")
        nc.vector.tensor_tensor(sq9[:], VN[:, :, 0:3], VN[:, :, 0:3], OP.mult)
        nc.vector.tensor_tensor(ta[:, 0], sq9[:, :, 0], sq9[:, :, 1], OP.add)
        nc.vector.tensor_tensor(VN[:, :, 3], ta[:, 0], sq9[:, :, 2], OP.add)

        # select the largest-norm candidate (two rounds, batched 4-row select)
        m4 = pool.tile([P, 4, NSEG], mybir.dt.uint8, tag="mask4")
        best = pool.tile([P, 4, NSEG], F32, tag="best")
        nc.vector.tensor_tensor(
            m4[:], VN[:, 1, 3:4].broadcast_to([P, 4, NSEG]),
            VN[:, 0, 3:4].broadcast_to([P, 4, NSEG]), OP.is_gt)
        nc.vector.select(best[:], m4[:], VN[:, 1], VN[:, 0])
        nc.vector.tensor_tensor(
            m4[:], VN[:, 2, 3:4].broadcast_to([P, 4, NSEG]),
            best[:, 3:4].broadcast_to([P, 4, NSEG]), OP.is_gt)
        nc.vector.select(best[:], m4[:], VN[:, 2], best[:])

        # rn = 1/|e|^2 ; rsqn = sqrt(rn) = 1/|e|
        rn = pool.tile([P, 1, NSEG], F32, tag="rn")
        nc.vector.reciprocal(rn[:], best[:, 3:4])
        rsqn = pool.tile([P, 1, NSEG], F32, tag="rsqn")
        nc.scalar.activation(rsqn[:], rn[:], AF.Sqrt)
        eb = pool.tile([P, 3, NSEG], BF16, tag="eb")
        nc.vector.tensor_copy(eb[:], best[:, 0:3])
        rnb = pool.tile([P, 1, NSEG], BF16, tag="rnb")
        nc.vector.tensor_copy(rnb[:], rn[:])

        # ---------------- phase 2 tail: per segment-half --------------------
        sc = pool.tile([P, 2, HALF], F32, tag="sc")
        for h in range(2):
            sl = slice(h * HALF, (h + 1) * HALF)
            uh = sp.tile([P, V, HALF], BF16, tag="uh", name=f"uh{h}")
            # x0' = xc . e (unnormalized)
            nc.vector.tensor_tensor(x0h[h][:], X[:, :, sl], cbc(eb, 0, h),
                                    OP.mult)
            nc.vector.tensor_tensor(uh[:], Y[:, :, sl], cbc(eb, 1, h), OP.mult)
            nc.gpsimd.tensor_tensor(x0h[h][:], x0h[h][:], uh[:], OP.add)
            nc.vector.tensor_tensor(uh[:], Z[:, :, sl], cbc(eb, 2, h), OP.mult)
            nc.gpsimd.tensor_tensor(x0h[h][:], x0h[h][:], uh[:], OP.add)
            # h = r2 - x0'^2 * rn ; np0 = sqrt(max(h, 0))
            x0sq = sp.tile([P, V, HALF], BF16, tag="x0sq", name=f"x0sq{h}")
            nc.vector.tensor_tensor(x0sq[:], x0h[h][:], x0h[h][:], OP.mult)
            nc.gpsimd.tensor_tensor(x0sq[:], x0sq[:], cbc(rnb, 0, h), OP.mult)
            nc.vector.tensor_tensor(r2h[h][:], r2h[h][:], x0sq[:], OP.subtract)
            nc.vector.tensor_scalar(out=r2h[h][:], in0=r2h[h][:], scalar1=0.0,
                                    scalar2=None, op0=OP.max)
            nc.scalar.activation(r2h[h][:], r2h[h][:], AF.Sqrt)
            # pr = x0' * np0 ; tree-reduce over v -> sc
            nc.vector.tensor_tensor(x0h[h][:], x0h[h][:], r2h[h][:], OP.mult)
            pr = x0h[h]
            w = V
            while w > 8:
                w //= 2
                nc.vector.tensor_tensor(pr[:, 0:w], pr[:, 0:w],
                                        pr[:, w:2 * w], OP.add)
            nc.vector.tensor_reduce(
                sc[:, h], pr[:, 0:8].rearrange("p v s -> p s v"),
                axis=AX.X, op=OP.add)

        # ---------------- finalize: feats [P, NSEG, 16] ----------------
        feats = pool.tile([P, NSEG, 16], F32, tag="feats")
        sgn = pool.tile([P, 1, NSEG], F32, tag="sgn")
        nc.scalar.activation(sgn[:, 0], sc[:].rearrange("p h s -> p (h s)"),
                             AF.Sign)
        fac = pool.tile([P, 1, NSEG], F32, tag="fac")
        nc.vector.tensor_tensor(fac[:], dirwt[:], rsqn[:], OP.mult)
        nc.vector.tensor_tensor(fac[:], fac[:], sgn[:], OP.mult)

        nc.gpsimd.tensor_copy(
            feats[:, :, 0:3].rearrange("p s k -> p k s"), c3[:])
        bb = pool.tile([P, 6, NSEG], F32, tag="bb")
        nc.vector.tensor_tensor(
            bb[:], amat[:], invw3[:].broadcast_to([P, 6, NSEG]), OP.mult)
        for col, row in ((3, 0), (4, 3), (5, 4), (6, 3), (7, 1), (8, 5),
                         (9, 4), (10, 5), (11, 2)):
            nc.gpsimd.tensor_copy(feats[:, :, col], bb[:, row])
        v0t = pool.tile([P, 3, NSEG], F32, tag="v0t")
        nc.vector.tensor_tensor(
            v0t[:], best[:, 0:3], fac[:].broadcast_to([P, 3, NSEG]), OP.mult)
        nc.gpsimd.tensor_copy(
            feats[:, :, 12:15].rearrange("p s k -> p k s"), v0t[:])
        size_t = pool.tile([P, NSEG], F32, tag="size_t")
        nc.gpsimd.memset(size_t[:], float(V))
        nc.gpsimd.tensor_copy(feats[:, :, 15], size_t[:])

        nc.sync.dma_start(feats_d.rearrange("(p s) k -> p s k", p=P), feats[:])

    if not nc.is_finalized():
        nc.finalize()
    return nc


def kernel(data: np.ndarray, clusts: np.ndarray) -> np.ndarray:
    import ml_dtypes
    BF = ml_dtypes.bfloat16

    data = np.asarray(data, dtype=np.float32)
    clusts_np = np.asarray(clusts)
    C, S = clusts_np.shape
    assert (C, S) == (N_CLUSTS, CLUST_SIZE), (C, S)

    vox = data[:, 1:4]
    g = vox[clusts_np.reshape(-1).astype(np.int64)].reshape(C, S, 3)
    g = g.astype(BF)

    if "nc" not in _CACHED:
        _CACHED["nc"] = build_nc()
    nc = _CACHED["nc"]

    in_maps = []
    for c in range(N_CORES):
        sl = slice(c * C_LOC, (c + 1) * C_LOC)
        gc = g[sl]  # [C_LOC, S, 3]
        m = {}
        for i, n in enumerate("xyz"):
            a4 = gc[:, :, i].reshape(P, NSEG, V)
            # cluster-major segment-minor [P, V, NSEG]
            m[n] = np.ascontiguousarray(a4.transpose(0, 2, 1)).reshape(
                P, V * NSEG)
            # voxel-major [V, C_LOC], column s*128+p = cluster p*32+s
            m[n + "t"] = np.ascontiguousarray(
                a4.transpose(2, 1, 0).reshape(V, C_LOC))
        in_maps.append(m)

    kw = {}
    if PROFILE:
        kw = dict(trace=True)
    res = run_bass_kernel_spmd(nc, in_maps, list(range(N_CORES)), **kw)
    if PROFILE:
        global LAST_RESULT
        LAST_RESULT = res
    out = np.concatenate([res.results[c]["feats"] for c in range(N_CORES)],
                         axis=0)
    return out.astype(np.float32)


# revision 21
# speedup vs baseline: 1.9704x; 1.2714x over previous
"""Trainium2 Bass kernel: per-cluster PCA geometry features (segment reduce).

Problem: data [4194304, 6] f32, clusts [32768, 128] int — per cluster of 128
voxels compute: center (mean of xyz), normalized covariance B = A/lmax,
principal axis v0 scaled by dirwt = 1 - lmid/lmax with a sign fix, size.

v2 design (cost-model driven):
- 32768 clusters sharded over 8 cores (4096 each); cluster c = p*32 + s
  (partition p, segment s). All plane data in bf16.
- Dual layout per coordinate: voxel-major [V, C_LOC] for PE column sums
  (phase-1 moments), cluster-major *segment-minor* [P, V, S] so per-cluster
  scalar broadcasts keep a packed (stride-1) last dim -> DVE 2x mode.
- Phase 1: squares on ACT, cross products on GPSIMD, sums via 288 tiny
  PE matmuls into one PSUM tile.
- Eigensolve: batched small-tile ops on [128, K, 32] tiles (trig method),
  eigenvector via row-cross-products on a doubled-rows tile so the three
  candidate cross products are affine-sliced batched ops.
- Phase 2 (per segment-half, pipelined): xc = X - c, r2 = |xc|^2 via ACT
  squares + GPSIMD adds, x0' = xc . e (unnormalized eigvec, avoids a sqrt
  on the critical path), h = r2 - x0'^2/|e|^2, np0 = ACT sqrt of clamped h,
  sign criterion sc = sum_v x0'*np0 via packed tree-reduction.
"""
import numpy as np
from contextlib import ExitStack

import concourse.bass as bass
import concourse.bacc as bacc
import concourse.tile as tile
from concourse import mybir
from concourse.bass_utils import run_bass_kernel_spmd

N_CLUSTS = 32768
CLUST_SIZE = 128
N_CORES = 8
C_LOC = N_CLUSTS // N_CORES   # 4096 clusters per core
P = 128                       # SBUF partitions
NSEG = C_LOC // P             # 32 clusters (segments) per partition
V = CLUST_SIZE                # 128 voxels per cluster
HALF = NSEG // 2

F32 = mybir.dt.float32
BF16 = mybir.dt.bfloat16
AF = mybir.ActivationFunctionType
OP = mybir.AluOpType
AX = mybir.AxisListType

_CACHED = {}
PROFILE = False          # set by test.py to capture an NTFF trace
LAST_RESULT = None       # BassKernelResults of the last run when PROFILE
INPUT_SHAPES = {
    "x": (P, V * NSEG), "y": (P, V * NSEG), "z": (P, V * NSEG),
    "xt": (V, C_LOC), "yt": (V, C_LOC), "zt": (V, C_LOC),
}
INPUT_DTYPE = "bfloat16"


def build_nc():
    nc = bacc.Bacc()
    # cluster-major segment-minor: row p holds [v, s] (v outer, s inner)
    x_d = nc.dram_tensor("x", [P, V * NSEG], BF16, kind="ExternalInput").ap()
    y_d = nc.dram_tensor("y", [P, V * NSEG], BF16, kind="ExternalInput").ap()
    z_d = nc.dram_tensor("z", [P, V * NSEG], BF16, kind="ExternalInput").ap()
    # voxel-major: column s*128+p holds cluster p*32+s
    xt_d = nc.dram_tensor("xt", [V, C_LOC], BF16, kind="ExternalInput").ap()
    yt_d = nc.dram_tensor("yt", [V, C_LOC], BF16, kind="ExternalInput").ap()
    zt_d = nc.dram_tensor("zt", [V, C_LOC], BF16, kind="ExternalInput").ap()
    feats_d = nc.dram_tensor("feats", [C_LOC, 16], F32, kind="ExternalOutput").ap()

    with tile.TileContext(nc) as tc, ExitStack() as ctx:
        pool = ctx.enter_context(tc.tile_pool(name="main", bufs=1))
        sp = ctx.enter_context(tc.tile_pool(name="scratch", bufs=6))
        spc = ctx.enter_context(tc.tile_pool(name="scratchc", bufs=4))
        spt = ctx.enter_context(tc.tile_pool(name="scratcht", bufs=2))
        pp = ctx.enter_context(tc.tile_pool(name="psum", bufs=1, space="PSUM"))

        ones = pool.tile([P, 1], BF16, tag="ones")
        nc.gpsimd.memset(ones[:], 1.0)
        warm = pool.tile([P, 1], F32, tag="warm")
        nc.gpsimd.memset(warm[:], 1.0)
        nc.scalar.activation(warm[:], warm[:], AF.Sqrt)

        # ---------------- input DMAs ----------------
        Xt = pool.tile([P, C_LOC], BF16, tag="Xt")
        Yt = pool.tile([P, C_LOC], BF16, tag="Yt")
        Zt = pool.tile([P, C_LOC], BF16, tag="Zt")
        X = pool.tile([P, V, NSEG], BF16, tag="X")
        Y = pool.tile([P, V, NSEG], BF16, tag="Y")
        Z = pool.tile([P, V, NSEG], BF16, tag="Z")
        nc.sync.dma_start(Yt[:], yt_d)
        nc.sync.dma_start(Zt[:], zt_d)
        nc.sync.dma_start(Xt[:], xt_d)
        nc.sync.dma_start(X[:], x_d.rearrange("p (v s) -> p v s", s=NSEG))
        nc.sync.dma_start(Y[:], y_d.rearrange("p (v s) -> p v s", s=NSEG))
        nc.sync.dma_start(Z[:], z_d.rearrange("p (v s) -> p v s", s=NSEG))

        # ---------------- phase 1: moments via PE column sums ----------------
        psum = pp.tile([P, 9, NSEG], F32, tag="psums")

        def pe_colsums(plane_t, k):
            for j in range(NSEG):
                nc.tensor.matmul(
                    out=psum[:, k, j : j + 1],
                    lhsT=plane_t[:, j * P : (j + 1) * P],
                    rhs=ones[:, 0:1],
                    start=True,
                    stop=True,
                )

        pe_colsums(Yt, 1)
        sq2 = sp.tile([P, C_LOC], BF16, tag="vplane", name="sq2")
        nc.scalar.activation(sq2[:], Yt[:], AF.Square)
        pe_colsums(sq2, 4)
        pe_colsums(Zt, 2)
        sq3 = sp.tile([P, C_LOC], BF16, tag="vplane", name="sq3")
        nc.scalar.activation(sq3[:], Zt[:], AF.Square)
        pe_colsums(sq3, 5)
        pe_colsums(Xt, 0)
        mom = pool.tile([P, 9, NSEG], F32, tag="mom")
        nc.vector.tensor_copy(mom[:, 0:3], psum[:, 0:3])
        inv_s = 1.0 / V
        c3 = pool.tile([P, 3, NSEG], F32, tag="c3")
        nc.vector.tensor_scalar(out=c3[:], in0=mom[:, 0:3], scalar1=inv_s,
                                scalar2=None, op0=OP.mult)
        cb = pool.tile([P, 3, NSEG], BF16, tag="cb")
        nc.vector.tensor_copy(cb[:], c3[:])
        pr3 = sp.tile([P, C_LOC], BF16, tag="vplane", name="pr3")
        nc.gpsimd.tensor_tensor(pr3[:], Yt[:], Zt[:], OP.mult)
        pe_colsums(pr3, 8)
        sq1 = sp.tile([P, C_LOC], BF16, tag="vplane", name="sq1")
        nc.vector.tensor_tensor(sq1[:], Xt[:], Xt[:], OP.mult)
        pe_colsums(sq1, 3)
        pr1 = sp.tile([P, C_LOC], BF16, tag="vplane", name="pr1")
        nc.gpsimd.tensor_tensor(pr1[:], Xt[:], Yt[:], OP.mult)
        pe_colsums(pr1, 6)
        pr2 = sp.tile([P, C_LOC], BF16, tag="vplane", name="pr2")
        nc.vector.tensor_tensor(pr2[:], Xt[:], Zt[:], OP.mult)
        pe_colsums(pr2, 7)

        # second moments: rows 3-8 = [Mxx Myy Mzz Mxy Mxz Myz]
        nc.vector.tensor_copy(mom[:, 3:9], psum[:, 3:9])

        def bcast_v(t, lo, hi, sl, n=1):
            # [P, K, NSEG] rows lo:hi, segments sl -> [P, n, V, W] broadcast
            w = sl.stop - sl.start
            if n == 1:
                return t[:, lo:hi, sl].broadcast_to([P, V, w]) if hi - lo == 1 \
                    else None
            return None

        # a = M - S*c ; amat rows: [axx, ayy, azz, axy, axz, ayz]
        amat = pool.tile([P, 6, NSEG], F32, tag="amat")
        t6 = pool.tile([P, 6, NSEG], F32, tag="t6")
        nc.vector.tensor_tensor(t6[:, 0:3], mom[:, 0:3], c3[:], OP.mult)
        nc.vector.tensor_tensor(
            t6[:, 3:5], mom[:, 0:1].broadcast_to([P, 2, NSEG]),
            c3[:, 1:3], OP.mult)
        nc.vector.tensor_tensor(t6[:, 5:6], mom[:, 1:2], c3[:, 2:3], OP.mult)
        nc.vector.tensor_tensor(amat[:], mom[:, 3:9], t6[:], OP.subtract)

        # ---- phase 2 state ------------------------------------------------
        planes = [X, Y, Z]
        r2h = [pool.tile([P, V, HALF], BF16, tag=f"r2h{h}", name=f"r2h{h}")
               for h in range(2)]
        x0h = [pool.tile([P, V, HALF], BF16, tag=f"x0h{h}", name=f"x0h{h}")
               for h in range(2)]
        sqc = [[spc.tile([P, V, HALF], BF16, tag="sqc", name=f"sqc{i}{h}")
                for h in range(2)] for i in range(3)]

        def cbc(t, row, h, w=HALF):
            # [P, K, NSEG] row -> [P, V, w] broadcast over voxels
            return t[:, row:row + 1, h * HALF:h * HALF + w].broadcast_to(
                [P, V, w])

        def xc_i(i, eng=None):
            # center one coordinate (both halves)
            eng = eng or (nc.vector if i == 0 else nc.gpsimd)
            pl = planes[i]
            for h in range(2):
                sl = slice(h * HALF, (h + 1) * HALF)
                eng.tensor_tensor(pl[:, :, sl], pl[:, :, sl],
                                  cbc(cb, i, h), OP.subtract)

        def sqc_i(i):
            # square one centered coordinate (both halves) on ACT
            pl = planes[i]
            for h in range(2):
                sl = slice(h * HALF, (h + 1) * HALF)
                nc.scalar.activation(sqc[i][h][:], pl[:, :, sl], AF.Square)

        # a = M - S*c ; amat rows: [axx, ayy, azz, axy, axz, ayz]
        amat = pool.tile([P, 6, NSEG], F32, tag="amat")
        t6 = pool.tile([P, 6, NSEG], F32, tag="t6")
        nc.vector.tensor_tensor(t6[:, 0:3], mom[:, 0:3], c3[:], OP.mult)
        nc.vector.tensor_tensor(
            t6[:, 3:5], mom[:, 0:1].broadcast_to([P, 2, NSEG]),
            c3[:, 1:3], OP.mult)
        nc.vector.tensor_tensor(t6[:, 5:6], mom[:, 1:2], c3[:, 2:3], OP.mult)
        nc.vector.tensor_tensor(amat[:], mom[:, 3:9], t6[:], OP.subtract)

        # ---------------- eigensolve (batched [P, K, NSEG] f32) -------------
        q = pool.tile([P, 1, NSEG], F32, tag="q")
        t1a = pool.tile([P, 1, NSEG], F32, tag="t1a")
        t1b = pool.tile([P, 1, NSEG], F32, tag="t1b")
        nc.vector.tensor_tensor(t1a[:], amat[:, 0:1], amat[:, 1:2], OP.add)
        nc.vector.tensor_tensor(t1a[:], t1a[:], amat[:, 2:3], OP.add)
        nc.vector.tensor_scalar(out=q[:], in0=t1a[:], scalar1=1.0 / 3.0,
                                scalar2=None, op0=OP.mult)
        # ba rows: [b11, b22, b33, axy, axz, ayz]  (traceless part)
        ba = pool.tile([P, 6, NSEG], F32, tag="ba")
        nc.vector.tensor_tensor(
            ba[:, 0:3], amat[:, 0:3], q[:].broadcast_to([P, 3, NSEG]),
            OP.subtract)
        nc.vector.tensor_copy(ba[:, 3:6], amat[:, 3:6])

        # p2 = sum(bd^2) + 2*sum(off^2) ; p = sqrt(p2/6); invp = 1/p
        sq6 = pool.tile([P, 6, NSEG], F32, tag="sq6")
        nc.vector.tensor_tensor(sq6[:], ba[:], ba[:], OP.mult)
        w3t = pool.tile([P, 3, NSEG], F32, tag="w3t")
        nc.vector.scalar_tensor_tensor(out=w3t[:], in0=sq6[:, 3:6], scalar=2.0,
                                       in1=sq6[:, 0:3], op0=OP.mult, op1=OP.add)
        p2s = pool.tile([P, 1, NSEG], F32, tag="p2s")
        nc.vector.tensor_tensor(t1a[:], w3t[:, 0:1], w3t[:, 1:2], OP.add)
        nc.vector.tensor_tensor(p2s[:], t1a[:], w3t[:, 2:3], OP.add)
        p_t = pool.tile([P, 1, NSEG], F32, tag="p_t")
        nc.scalar.activation(p_t[:], p2s[:], AF.Sqrt, scale=1.0 / 6.0)
        invp = pool.tile([P, 1, NSEG], F32, tag="invp")
        nc.vector.reciprocal(invp[:], p_t[:])

        # center + square x (DVE was busy with the eigen chain until now)
        xc_i(0)
        sqc_i(0)
        xc_i(1)

        # det of the raw traceless matrix ba on DVE (overlaps the ACT sqrt
        # for p); r = det(ba) * invp^3 / 2, clamped to [-1, 1].
        # minors vs row1: m1 = b22*b33 - b23^2 ; m2 = b12*b33 - b23*b13 ;
        # m3 = b12*b23 - b22*b13 ; det = b11*m1 - b12*m2 + b13*m3
        r_t = pool.tile([P, 1, NSEG], F32, tag="r_t")
        det_a = pool.tile([P, 3, NSEG], F32, tag="det_a")
        det_b = pool.tile([P, 3, NSEG], F32, tag="det_b")
        nc.vector.tensor_tensor(det_a[:, 0:1], ba[:, 1:2], ba[:, 2:3],
                                OP.mult)
        nc.vector.tensor_tensor(
            det_a[:, 1:3], ba[:, 3:4].broadcast_to([P, 2, NSEG]),
            ba[:, 2:6:3], OP.mult)
        nc.vector.tensor_tensor(det_b[:, 0:1], ba[:, 5:6], ba[:, 5:6],
                                OP.mult)
        nc.vector.tensor_tensor(det_b[:, 1:2], ba[:, 5:6], ba[:, 4:5],
                                OP.mult)
        nc.vector.tensor_tensor(det_b[:, 2:3], ba[:, 1:2], ba[:, 4:5],
                                OP.mult)
        nc.vector.tensor_tensor(det_a[:], det_a[:], det_b[:], OP.subtract)
        nc.vector.tensor_tensor(det_b[:, 0:1], ba[:, 0:1], det_a[:, 0:1],
                                OP.mult)
        nc.vector.tensor_tensor(det_b[:, 1:3], ba[:, 3:5], det_a[:, 1:3],
                                OP.mult)
        nc.vector.tensor_tensor(t1b[:], det_b[:, 0:1], det_b[:, 1:2],
                                OP.subtract)
        nc.vector.tensor_tensor(t1b[:], t1b[:], det_b[:, 2:3], OP.add)
        # r = det * invp^3 / 2, clamped
        nc.vector.tensor_tensor(t1a[:], invp[:], invp[:], OP.mult)
        nc.vector.tensor_tensor(t1a[:], t1a[:], invp[:], OP.mult)
        nc.vector.tensor_tensor(t1b[:], t1b[:], t1a[:], OP.mult)
        nc.vector.tensor_scalar(out=r_t[:], in0=t1b[:], scalar1=0.5,
                                scalar2=1.0, op0=OP.mult, op1=OP.min)
        nc.vector.tensor_scalar(out=r_t[:], in0=r_t[:], scalar1=-1.0,
                                scalar2=None, op0=OP.max)

        # square y on ACT; center z on GPSIMD (z DMA lands about now)
        sqc_i(1)
        xc_i(2)

        # f = cos(acos(r)/3): largest root of 4f^3 - 3f = r.
        # Chebyshev deg-8 seed (err 2.2e-2) + 2 Newton steps -> 5.5e-3 max
        # (only near r=-1, where the top eigenpair degenerates and dirwt -> 0).
        SEED = [0.8649279174994734, 0.15532929881670984, -0.0021054445875550026,
                0.14553392157332898, -0.3069890177054866, -0.3028838631742596,
                0.5725439670593226, 0.24058719928428143, -0.3682048402442527]
        f_t = pool.tile([P, 1, NSEG], F32, tag="f_t")
        nc.vector.tensor_scalar(out=f_t[:], in0=r_t[:], scalar1=0.0,
                                scalar2=SEED[8], op0=OP.mult, op1=OP.add)
        for k in range(8, 0, -1):
            # f <- (f + a_k) * r, then finally + a_0
            nc.vector.scalar_tensor_tensor(
                out=f_t[:], in0=f_t[:], scalar=0.0 if k == 8 else SEED[k],
                in1=r_t[:], op0=OP.add, op1=OP.mult)
        nc.vector.tensor_scalar(out=f_t[:], in0=f_t[:], scalar1=SEED[0],
                                scalar2=None, op0=OP.add)
        nc.vector.tensor_scalar(out=f_t[:], in0=f_t[:], scalar1=0.5,
                                scalar2=1.0, op0=OP.max, op1=OP.min)
        f2 = pool.tile([P, 1, NSEG], F32, tag="f2")
        for _ in range(1):
            nc.vector.tensor_tensor(f2[:], f_t[:], f_t[:], OP.mult)
            nc.vector.tensor_scalar(out=t1a[:], in0=f2[:], scalar1=12.0,
                                    scalar2=-3.0, op0=OP.mult, op1=OP.add)
            nc.vector.tensor_scalar(out=t1a[:], in0=t1a[:], scalar1=1e-3,
                                    scalar2=None, op0=OP.max)
            nc.vector.reciprocal(t1a[:], t1a[:])
            nc.vector.tensor_scalar(out=t1b[:], in0=f2[:], scalar1=4.0,
                                    scalar2=-3.0, op0=OP.mult, op1=OP.add)
            nc.vector.tensor_tensor(t1b[:], t1b[:], f_t[:], OP.mult)
            nc.vector.tensor_tensor(t1b[:], t1b[:], r_t[:], OP.subtract)
            nc.vector.tensor_tensor(t1b[:], t1b[:], t1a[:], OP.mult)
            nc.vector.tensor_tensor(f_t[:], f_t[:], t1b[:], OP.subtract)
            nc.vector.tensor_scalar(out=f_t[:], in0=f_t[:], scalar1=0.5,
                                    scalar2=1.0, op0=OP.max, op1=OP.min)

        # w3 = q + 2*p*f ; w2 = q + (sqrt(2*p2*(1-f^2)) - 2*p*f)/2
        # (w2 from the deflated quadratic t^2 + t3 t + t3^2 - p2/2 = 0)
        w3 = pool.tile([P, 1, NSEG], F32, tag="w3")
        w2 = pool.tile([P, 1, NSEG], F32, tag="w2")
        m1 = pool.tile([P, 1, NSEG], F32, tag="m1")
        nc.vector.tensor_tensor(m1[:], p_t[:], f_t[:], OP.mult)   # p*f
        nc.vector.scalar_tensor_tensor(out=w3[:], in0=m1[:], scalar=2.0,
                                       in1=q[:], op0=OP.mult, op1=OP.add)
        nc.vector.tensor_tensor(f2[:], f_t[:], f_t[:], OP.mult)
        nc.vector.tensor_scalar(out=t1b[:], in0=f2[:], scalar1=-1.0,
                                scalar2=1.0, op0=OP.mult, op1=OP.add)
        nc.vector.scalar_tensor_tensor(out=t1b[:], in0=p2s[:], scalar=2.0,
                                       in1=t1b[:], op0=OP.mult, op1=OP.mult)
        nc.scalar.activation(t1a[:], t1b[:], AF.Sqrt)   # sqrt(disc)
        nc.vector.scalar_tensor_tensor(out=t1b[:], in0=t1a[:], scalar=0.5,
                                       in1=m1[:], op0=OP.mult, op1=OP.subtract)
        nc.vector.tensor_tensor(w2[:], t1b[:], q[:], OP.add)

        invw3 = pool.tile([P, 1, NSEG], F32, tag="invw3")
        nc.vector.reciprocal(invw3[:], w3[:])
        dirwt = pool.tile([P, 1, NSEG], F32, tag="dirwt")
        nc.vector.tensor_tensor(dirwt[:], w2[:], invw3[:], OP.mult)
        nc.vector.tensor_scalar(out=dirwt[:], in0=dirwt[:], scalar1=-1.0,
                                scalar2=1.0, op0=OP.mult, op1=OP.add)

        # square z on ACT; r2 = sum of squares on GPSIMD
        sqc_i(2)
        for h, eng in ((0, nc.vector), (1, nc.gpsimd)):
            eng.tensor_tensor(r2h[h][:], sqc[0][h][:], sqc[1][h][:], OP.add)
            eng.tensor_tensor(r2h[h][:], r2h[h][:], sqc[2][h][:], OP.add)

        # ---- eigenvector: batched row cross products -------------------
        # rows of (A - w3 I): r1 = (d1, axy, axz), r2 = (axy, d2, ayz),
        # r3 = (axz, ayz, d3); D = per-vector doubled rows.
        rows = pool.tile([P, 9, NSEG], F32, tag="rows")
        nc.vector.tensor_tensor(
            rows[:, 0:9:4], amat[:, 0:3],
            w3[:].broadcast_to([P, 3, NSEG]), OP.subtract)
        nc.vector.tensor_copy(rows[:, 1:2], amat[:, 3:4])   # axy
        nc.vector.tensor_copy(rows[:, 2:3], amat[:, 4:5])   # axz
        nc.vector.tensor_copy(rows[:, 3:4], amat[:, 3:4])   # axy
        nc.vector.tensor_copy(rows[:, 5:6], amat[:, 5:6])   # ayz
        nc.vector.tensor_copy(rows[:, 6:7], amat[:, 4:5])   # axz
        nc.vector.tensor_copy(rows[:, 7:8], amat[:, 5:6])   # ayz
        D = pool.tile([P, 3, 2, 3, NSEG], F32, tag="D")
        rows_g = rows[:].rearrange("p (g c) s -> p g c s", g=3)
        nc.vector.tensor_copy(D[:, :, 0], rows_g)
        nc.vector.tensor_copy(D[:, :, 1], rows_g)
        Dg = D[:].rearrange("p g r c s -> p g (r c) s")  # [P, 3, 6, NSEG]

        # VN rows: cand_a(0:3), na(3), cand_b(4:7), nb(7), cand_c(8:11), nc(11)
        VN = pool.tile([P, 3, 4, NSEG], F32, tag="VN")
        ta = pool.tile([P, 2, 3, NSEG], F32, tag="ta")
        tb = pool.tile([P, 2, 3, NSEG], F32, tag="tb")
        r1s1 = Dg[:, 0:1, 1:4].broadcast_to([P, 2, 3, NSEG])
        r1s2 = Dg[:, 0:1, 2:5].broadcast_to([P, 2, 3, NSEG])
        nc.vector.tensor_tensor(ta[:], r1s1, Dg[:, 1:3, 2:5], OP.mult)
        nc.vector.tensor_tensor(tb[:], r1s2, Dg[:, 1:3, 1:4], OP.mult)
        nc.vector.tensor_tensor(ta[:], ta[:], tb[:], OP.subtract)
        nc.vector.tensor_copy(VN[:, 0:2, 0:3], ta[:])
        nc.vector.tensor_tensor(tb[:, 0], Dg[:, 1, 1:4], Dg[:, 2, 2:5], OP.mult)
        nc.vector.tensor_tensor(tb[:, 1], Dg[:, 1, 2:5], Dg[:, 2, 1:4], OP.mult)
        nc.vector.tensor_tensor(VN[:, 2, 0:3], tb[:, 0], tb[:, 1], OP.subtract)

        # norms into VN[:, :, 3]
        sq9 = pool.tile([P, 3, 3, NSEG], F32, tag="sq9")
        nc.vector.tensor_tensor(sq9[:], VN[:, :, 0:3], VN[:, :, 0:3], OP.mult)
        nc.vector.tensor_tensor(ta[:, 0], sq9[:, :, 0], sq9[:, :, 1], OP.add)
        nc.vector.tensor_tensor(VN[:, :, 3], ta[:, 0], sq9[:, :, 2], OP.add)

        # select the largest-norm candidate (two rounds, batched 4-row select)
        m4 = pool.tile([P, 4, NSEG], mybir.dt.uint8, tag="mask4")
        best = pool.tile([P, 4, NSEG], F32, tag="best")
        nc.vector.tensor_tensor(
            m4[:], VN[:, 1, 3:4].broadcast_to([P, 4, NSEG]),
            VN[:, 0, 3:4].broadcast_to([P, 4, NSEG]), OP.is_gt)
        nc.vector.select(best[:], m4[:], VN[:, 1], VN[:, 0])
        nc.vector.tensor_tensor(
            m4[:], VN[:, 2, 3:4].broadcast_to([P, 4, NSEG]),
            best[:, 3:4].broadcast_to([P, 4, NSEG]), OP.is_gt)
        nc.vector.select(best[:], m4[:], VN[:, 2], best[:])

        # rsqn = 1/|e| ; normalized eigvec in bf16
        rn = pool.tile([P, 1, NSEG], F32, tag="rn")
        nc.vector.reciprocal(rn[:], best[:, 3:4])
        rsqn = pool.tile([P, 1, NSEG], F32, tag="rsqn")
        nc.scalar.activation(rsqn[:], rn[:], AF.Sqrt)
        en = pool.tile([P, 3, NSEG], F32, tag="en")
        nc.vector.tensor_tensor(
            en[:], best[:, 0:3], rsqn[:].broadcast_to([P, 3, NSEG]), OP.mult)
        eb = pool.tile([P, 3, NSEG], BF16, tag="eb")
        nc.vector.tensor_copy(eb[:], en[:])

        # ---------------- phase 2 tail: per segment-half --------------------
        # h0's chain runs on DVE, h1's front half on GPSIMD, so the two
        # halves pipeline across engines; np0 sqrts on ACT.
        sc = pool.tile([P, 2, HALF], F32, tag="sc")
        uhs, u2s, xqs = [], [], []
        for h in range(2):
            uhs.append(spt.tile([P, V, HALF], BF16, tag="uh", name=f"uh{h}"))
            u2s.append(spt.tile([P, V, HALF], BF16, tag="uh2", name=f"uh2{h}"))
            xqs.append(spt.tile([P, V, HALF], BF16, tag="x0sq",
                                name=f"x0sq{h}"))
        for h, eng in ((0, nc.vector), (1, nc.gpsimd)):
            sl = slice(h * HALF, (h + 1) * HALF)
            # x0 = xc . e (normalized)
            eng.tensor_tensor(x0h[h][:], X[:, :, sl], cbc(eb, 0, h), OP.mult)
            eng.tensor_tensor(uhs[h][:], Y[:, :, sl], cbc(eb, 1, h), OP.mult)
            eng.tensor_tensor(u2s[h][:], Z[:, :, sl], cbc(eb, 2, h), OP.mult)
            eng.tensor_tensor(x0h[h][:], x0h[h][:], uhs[h][:], OP.add)
            eng.tensor_tensor(x0h[h][:], x0h[h][:], u2s[h][:], OP.add)
        for h in range(2):
            # h = r2 - x0^2 ; np0 = sqrt(max(h, 0)) ; pr = x0*np0 ; tree-sum
            nc.vector.tensor_tensor(xqs[h][:], x0h[h][:], x0h[h][:], OP.mult)
            nc.vector.tensor_tensor(r2h[h][:], r2h[h][:], xqs[h][:],
                                    OP.subtract)
            nc.vector.tensor_scalar(out=r2h[h][:], in0=r2h[h][:], scalar1=0.0,
                                    scalar2=None, op0=OP.max)
            nc.scalar.activation(r2h[h][:], r2h[h][:], AF.Sqrt)
            nc.vector.tensor_tensor(x0h[h][:], x0h[h][:], r2h[h][:], OP.mult)
            pr = x0h[h]
            w = V
            while w > 8:
                w //= 2
                nc.vector.tensor_tensor(pr[:, 0:w], pr[:, 0:w],
                                        pr[:, w:2 * w], OP.add)
            nc.vector.tensor_reduce(
                sc[:, h], pr[:, 0:8].rearrange("p v s -> p s v"),
                axis=AX.X, op=OP.add)

        # ---------------- finalize: feats [P, NSEG, 16] ----------------
        feats = pool.tile([P, NSEG, 16], F32, tag="feats")
        sgn = pool.tile([P, 1, NSEG], F32, tag="sgn")
        nc.scalar.activation(sgn[:, 0], sc[:].rearrange("p h s -> p (h s)"),
                             AF.Sign)
        fac = pool.tile([P, 1, NSEG], F32, tag="fac")
        nc.vector.tensor_tensor(fac[:], dirwt[:], rsqn[:], OP.mult)
        nc.vector.tensor_tensor(fac[:], fac[:], sgn[:], OP.mult)

        nc.gpsimd.tensor_copy(
            feats[:, :, 0:3].rearrange("p s k -> p k s"), c3[:])
        bb = pool.tile([P, 6, NSEG], F32, tag="bb")
        nc.vector.tensor_tensor(
            bb[:], amat[:], invw3[:].broadcast_to([P, 6, NSEG]), OP.mult)
        for col, row in ((3, 0), (4, 3), (5, 4), (6, 3), (7, 1), (8, 5),
                         (9, 4), (10, 5), (11, 2)):
            nc.gpsimd.tensor_copy(feats[:, :, col], bb[:, row])
        v0t = pool.tile([P, 3, NSEG], F32, tag="v0t")
        nc.vector.tensor_tensor(
            v0t[:], best[:, 0:3], fac[:].broadcast_to([P, 3, NSEG]), OP.mult)
        nc.gpsimd.tensor_copy(
            feats[:, :, 12:15].rearrange("p s k -> p k s"), v0t[:])
        size_t = pool.tile([P, NSEG], F32, tag="size_t")
        nc.gpsimd.memset(size_t[:], float(V))
        nc.gpsimd.tensor_copy(feats[:, :, 15], size_t[:])

        nc.sync.dma_start(feats_d.rearrange("(p s) k -> p s k", p=P), feats[:])

    if not nc.is_finalized():
        nc.finalize()
    return nc


def kernel(data: np.ndarray, clusts: np.ndarray) -> np.ndarray:
    import ml_dtypes
    BF = ml_dtypes.bfloat16

    data = np.asarray(data, dtype=np.float32)
    clusts_np = np.asarray(clusts)
    C, S = clusts_np.shape
    assert (C, S) == (N_CLUSTS, CLUST_SIZE), (C, S)

    vox = data[:, 1:4]
    g = vox[clusts_np.reshape(-1).astype(np.int64)].reshape(C, S, 3)
    g = g.astype(BF)

    if "nc" not in _CACHED:
        _CACHED["nc"] = build_nc()
    nc = _CACHED["nc"]

    in_maps = []
    for c in range(N_CORES):
        sl = slice(c * C_LOC, (c + 1) * C_LOC)
        gc = g[sl]  # [C_LOC, S, 3]
        m = {}
        for i, n in enumerate("xyz"):
            a4 = gc[:, :, i].reshape(P, NSEG, V)
            # cluster-major segment-minor [P, V, NSEG]
            m[n] = np.ascontiguousarray(a4.transpose(0, 2, 1)).reshape(
                P, V * NSEG)
            # voxel-major [V, C_LOC], column s*128+p = cluster p*32+s
            m[n + "t"] = np.ascontiguousarray(
                a4.transpose(2, 1, 0).reshape(V, C_LOC))
        in_maps.append(m)

    kw = {}
    if PROFILE:
        kw = dict(trace=True)
    res = run_bass_kernel_spmd(nc, in_maps, list(range(N_CORES)), **kw)
    if PROFILE:
        global LAST_RESULT
        LAST_RESULT = res
    out = np.concatenate([res.results[c]["feats"] for c in range(N_CORES)],
                         axis=0)
    return out.astype(np.float32)
